# revision 7
# baseline (speedup 1.0000x reference)
"""Trainium2 Bass kernel for nn_Attention (pooling attention head).

Reference computation (per batch b):
    score[t]  = hidden[t,:] @ W_score @ hidden[-1,:]        # via u = W_score @ h_t
    attn      = softmax(score)
    context   = sum_t attn[t] * hidden[t,:]
    out       = tanh(concat(context, h_t) @ W_out)

Key optimization: the reference computes (hidden @ W_score) [B,T,H] first
(69 GFLOP); we reassociate to u = W_score @ h_t (34 MFLOP) and then
score = hidden @ u, so the kernel is a single memory-bound streaming pass
over hidden_states.

Sharding: data-parallel over batch, 8 batches per NeuronCore, no
collectives. Each core returns its [8, 128] slice of the output.
"""

import os

os.environ.setdefault("MYCRO_LOCAL_CACHE", "1")

from contextlib import ExitStack

import numpy as np

import concourse.bass as bass
import concourse.bass_isa as bass_isa
import concourse.tile as tile
from concourse import bacc, mybir
from concourse.bass_utils import run_bass_kernel_spmd
from concourse.masks import make_identity

B, T, H, UNITS = 64, 2048, 512, 128
NCORES = 8
BL = B // NCORES  # local batches per core
NT = T // 128  # 16 t-tiles per batch

F32 = mybir.dt.float32
F32R = mybir.dt.float32r

# bisection/debug knobs (comma-separated in KVAR):
#   nopar   — replace gpsimd.partition_all_reduce with a (numerically wrong) copy
#   ttr     — use fused tensor_tensor_reduce (FAILS on HW via axon path)
#   nobcast — replace the step-0 broadcast DMA with a (wrong) plain DMA
KVAR = set(os.environ.get("KVAR", "").split(",")) - {""}


def _kernel_body(tc: tile.TileContext, out, hs, ws, wo, uT_dram):
    nc = tc.nc
    with ExitStack() as ctx:
        singles = ctx.enter_context(tc.tile_pool(name="singles", bufs=1))
        hid_pool = ctx.enter_context(tc.tile_pool(name="hid", bufs=2 * NT))
        work = ctx.enter_context(tc.tile_pool(name="work", bufs=2))
        small = ctx.enter_context(tc.tile_pool(name="small", bufs=2))
        ps_setup = ctx.enter_context(tc.tile_pool(name="ps_setup", bufs=2, space="PSUM"))
        ps_ctx = ctx.enter_context(tc.tile_pool(name="ps_ctx", bufs=2, space="PSUM"))

        ident = singles.tile([128, 128], F32)
        make_identity(nc, ident)

        # ---- load weights / last-timestep rows --------------------------
        ws_sb = singles.tile([128, 4, H], F32)  # W_score rows r*128+p
        nc.sync.dma_start(out=ws_sb, in_=ws.rearrange("(r p) k -> p r k", p=128))
        wout_sb = singles.tile([128, 8, UNITS], F32)  # W_out rows c*128+p
        nc.sync.dma_start(out=wout_sb, in_=wo.rearrange("(c p) j -> p c j", p=128))
        ht_sb = singles.tile([BL, H], F32)  # h_t = hidden[:, -1, :]
        nc.sync.dma_start(out=ht_sb, in_=hs[:, T - 1, :])

        # ---- W_score^T (PE transposes): wsT_sb[p, kc, h] = W_score[h, kc*128+p]
        wsT_sb = singles.tile([128, 4, H], F32)
        for r in range(4):
            for c in range(4):
                pst = ps_setup.tile([128, 128], F32, tag="setup")
                nc.tensor.transpose(pst, ws_sb[:, r, c * 128 : (c + 1) * 128], ident)
                nc.scalar.copy(wsT_sb[:, c, r * 128 : (r + 1) * 128], pst)

        # ---- h_t^T: htT_sb[p, c, b] = h_t[b, c*128+p]
        htT_sb = singles.tile([128, 4, BL], F32)
        for c in range(4):
            pst = ps_setup.tile([128, BL], F32, tag="setup")
            nc.tensor.transpose(
                pst, ht_sb[:, c * 128 : (c + 1) * 128], ident[:BL, :BL]
            )
            nc.scalar.copy(htT_sb[:, c, :], pst)

        # ---- u[b] = W_score @ h_t[b] for all local batches ---------------
        # u_sb[p, hc, b] = sum_k W_score[hc*128+p, k] * h_t[b, k]
        u_sb = singles.tile([128, 4, BL], F32)
        for hc in range(4):
            psu = ps_setup.tile([128, BL], F32, tag="setup")
            for kc in range(4):
                nc.tensor.matmul(
                    psu,
                    lhsT=wsT_sb[:, kc, hc * 128 : (hc + 1) * 128],
                    rhs=htT_sb[:, kc, :],
                    start=(kc == 0),
                    stop=(kc == 3),
                )
            nc.scalar.copy(u_sb[:, hc, :], psu)

        # stage u^T to DRAM so it can be partition-broadcast back per batch
        for hc in range(4):
            uT_ap = bass.AP(
                tensor=uT_dram.tensor, offset=hc * 128, ap=[[1, 128], [H, BL]]
            )
            nc.sync.dma_start(out=uT_ap, in_=u_sb[:, hc, :])

        # preT_sb[p, c, b]: transposed concat(context, h_t); ht half now
        preT_sb = singles.tile([128, 8, BL], F32)
        for c in range(4):
            nc.vector.tensor_copy(out=preT_sb[:, 4 + c, :], in_=htT_sb[:, c, :])

        # ---- main per-batch streaming loop ------------------------------
        for b in range(BL):
            # u[b] broadcast to all 128 partitions
            u_bc = work.tile([128, H], F32, tag="u_bc")
            if "nobcast" in KVAR:
                nc.sync.dma_start(
                    out=u_bc[0:BL, :], in_=uT_dram[:, :]
                )
            else:
                nc.gpsimd.dma_start(
                    out=u_bc,
                    in_=bass.AP(
                        tensor=uT_dram.tensor, offset=b * H, ap=[[0, 128], [1, H]]
                    ),
                )

            S = small.tile([128, NT], F32, tag="S")
            hids = []
            scratch = work.tile([128, H], F32, tag="scratch")
            for i in range(NT):
                hid = hid_pool.tile([128, H], F32, tag="hid")
                nc.sync.dma_start(out=hid, in_=hs[b, i * 128 : (i + 1) * 128, :])
                hids.append(hid)
                # score column: S[:, i] = sum_h hid * u
                if "ttr" not in KVAR:
                    nc.vector.tensor_mul(scratch, hid, u_bc)
                    nc.vector.reduce_sum(
                        S[:, i : i + 1], scratch, axis=mybir.AxisListType.X
                    )
                else:
                    nc.vector.tensor_tensor_reduce(
                        out=scratch,
                        in0=hid,
                        in1=u_bc,
                        scale=1.0,
                        scalar=0.0,
                        op0=mybir.AluOpType.mult,
                        op1=mybir.AluOpType.add,
                        accum_out=S[:, i : i + 1],
                    )

            # softmax over all 2048 scores of this batch
            m_row = small.tile([128, 1], F32, tag="m_row")
            nc.vector.reduce_max(m_row, S, axis=mybir.AxisListType.X)
            M_all = small.tile([128, 1], F32, tag="M_all")
            if "nopar" in KVAR:
                nc.vector.tensor_copy(out=M_all, in_=m_row)
            else:
                nc.gpsimd.partition_all_reduce(
                    M_all, m_row, channels=128, reduce_op=bass_isa.ReduceOp.max
                )
            nm = small.tile([128, 1], F32, tag="nm")
            nc.vector.tensor_scalar_mul(nm, M_all, -1.0)
            P = small.tile([128, NT], F32, tag="P")
            l_row = small.tile([128, 1], F32, tag="l_row")
            nc.scalar.activation(
                P,
                S,
                mybir.ActivationFunctionType.Exp,
                bias=nm,
                scale=1.0,
                accum_out=l_row,
            )
            L_all = small.tile([128, 1], F32, tag="L_all")
            if "nopar" in KVAR:
                nc.vector.tensor_copy(out=L_all, in_=l_row)
            else:
                nc.gpsimd.partition_all_reduce(
                    L_all, l_row, channels=128, reduce_op=bass_isa.ReduceOp.add
                )
            Linv = small.tile([128, 1], F32, tag="Linv")
            nc.vector.reciprocal(Linv, L_all)

            # context accumulation, already transposed:
            # psum_ctx[p, hc] = sum_t P[t] * hidden[t, hc*128+p]
            # (hidden tile as the stationary operand, P column as moving —
            # keeps fp32 precision and lands context in [h, b] layout)
            psum_ctx = ps_ctx.tile([128, 4], F32, tag="ctx")
            for hc in range(4):
                for i in range(NT):
                    nc.tensor.matmul(
                        psum_ctx[:, hc : hc + 1],
                        lhsT=hids[i][:, hc * 128 : (hc + 1) * 128],
                        rhs=P[:, i : i + 1],
                        start=(i == 0),
                        stop=(i == NT - 1),
                    )
            # normalize by 1/L and write into preT layout
            nc.vector.tensor_scalar_mul(preT_sb[:, 0:4, b], psum_ctx, Linv)

        # ---- final: out = tanh(pre @ W_out) -----------------------------
        psum_out = ps_setup.tile([BL, UNITS], F32, tag="setup")
        for c in range(8):
            nc.tensor.matmul(
                psum_out,
                lhsT=preT_sb[:, c, :],
                rhs=wout_sb[:, c, :],
                start=(c == 0),
                stop=(c == 7),
            )
        y_sb = small.tile([BL, UNITS], F32, tag="y")
        nc.scalar.activation(y_sb, psum_out, mybir.ActivationFunctionType.Tanh)
        nc.sync.dma_start(out=out, in_=y_sb)


def build_nc():
    nc = bacc.Bacc(
        "TRN2",
        target_bir_lowering=False,
        debug=False,
        enable_asserts=False,
        num_devices=NCORES,
    )
    hs = nc.dram_tensor(
        "hidden_states", [BL, T, H], F32, kind="ExternalInput"
    ).ap()
    ws = nc.dram_tensor("W_score", [H, H], F32, kind="ExternalInput").ap()
    wo = nc.dram_tensor("W_out", [2 * H, UNITS], F32, kind="ExternalInput").ap()
    out = nc.dram_tensor("out", [BL, UNITS], F32, kind="ExternalOutput").ap()
    uT_dram = nc.dram_tensor("uT_scratch", [BL, H], F32).ap()

    with tile.TileContext(nc) as tc:
        _kernel_body(tc, out, hs, ws, wo, uT_dram)
    nc.compile()
    return nc


_NC = None


def _get_nc():
    global _NC
    if _NC is None:
        _NC = build_nc()
    return _NC


def make_in_maps(hidden_states, W_score, W_out):
    hidden_states = np.ascontiguousarray(
        np.asarray(hidden_states, dtype=np.float32)
    )
    W_score = np.ascontiguousarray(np.asarray(W_score, dtype=np.float32))
    W_out = np.ascontiguousarray(np.asarray(W_out, dtype=np.float32))
    return [
        {
            "hidden_states": hidden_states[i * BL : (i + 1) * BL],
            "W_score": W_score,
            "W_out": W_out,
        }
        for i in range(NCORES)
    ]


def kernel(hidden_states, W_score, W_out):
    nc = _get_nc()
    in_maps = make_in_maps(hidden_states, W_score, W_out)
    res = run_bass_kernel_spmd(nc, in_maps, core_ids=list(range(NCORES)))
    return np.concatenate([res.results[i]["out"] for i in range(NCORES)], axis=0)


# revision 9
# speedup vs baseline: 1.5037x; 1.5037x over previous
"""Trainium2 Bass kernel for nn_Attention (pooling attention head).

Reference computation (per batch b):
    score[t]  = hidden[t,:] @ W_score @ hidden[-1,:]        # via u = W_score @ h_t
    attn      = softmax(score)
    context   = sum_t attn[t] * hidden[t,:]
    out       = tanh(concat(context, h_t) @ W_out)

Key optimization: the reference computes (hidden @ W_score) [B,T,H] first
(69 GFLOP); we reassociate to u = W_score @ h_t (34 MFLOP) and then
score = hidden @ u, so the kernel is a single memory-bound streaming pass
over hidden_states.

Sharding: data-parallel over batch, 8 batches per NeuronCore, no
collectives. Each core returns its [8, 128] slice of the output.
"""

import os

os.environ.setdefault("MYCRO_LOCAL_CACHE", "1")

from contextlib import ExitStack

import numpy as np

import concourse.bass as bass
import concourse.bass_isa as bass_isa
import concourse.tile as tile
from concourse import bacc, mybir
from concourse.bass_utils import run_bass_kernel_spmd
from concourse.masks import make_identity

B, T, H, UNITS = 64, 2048, 512, 128
NCORES = 8
BL = B // NCORES  # local batches per core
NT = T // 128  # 16 t-tiles per batch

F32 = mybir.dt.float32
F32R = mybir.dt.float32r
BF16 = mybir.dt.bfloat16

# bisection/debug knobs (comma-separated in KVAR):
#   nopar   — replace gpsimd.partition_all_reduce with a (numerically wrong) copy
#   ttr     — use fused tensor_tensor_reduce (FAILS on HW via axon path)
#   nobcast — replace the step-0 broadcast DMA with a (wrong) plain DMA
KVAR = set(os.environ.get("KVAR", "").split(",")) - {""}


def _kernel_body(tc: tile.TileContext, out, hs, ws, wo, uT_dram):
    nc = tc.nc
    with ExitStack() as ctx:
        singles = ctx.enter_context(tc.tile_pool(name="singles", bufs=1))
        hid_pool = ctx.enter_context(tc.tile_pool(name="hid", bufs=8))
        hid_bf_pool = ctx.enter_context(tc.tile_pool(name="hidbf", bufs=8))
        work = ctx.enter_context(tc.tile_pool(name="work", bufs=3))
        small = ctx.enter_context(tc.tile_pool(name="small", bufs=2))
        ps_setup = ctx.enter_context(tc.tile_pool(name="ps_setup", bufs=2, space="PSUM"))
        ps_ctx = ctx.enter_context(tc.tile_pool(name="ps_ctx", bufs=2, space="PSUM"))

        ident = singles.tile([128, 128], F32)
        make_identity(nc, ident)

        # ---- load weights / last-timestep rows --------------------------
        ws_sb = singles.tile([128, 4, H], F32)  # W_score rows r*128+p
        nc.sync.dma_start(out=ws_sb, in_=ws.rearrange("(r p) k -> p r k", p=128))
        wout_sb = singles.tile([128, 8, UNITS], F32)  # W_out rows c*128+p
        nc.sync.dma_start(out=wout_sb, in_=wo.rearrange("(c p) j -> p c j", p=128))
        ht_sb = singles.tile([BL, H], F32)  # h_t = hidden[:, -1, :]
        nc.sync.dma_start(out=ht_sb, in_=hs[:, T - 1, :])

        # ---- W_score^T (PE transposes): wsT_sb[p, kc, h] = W_score[h, kc*128+p]
        wsT_sb = singles.tile([128, 4, H], F32)
        for r in range(4):
            for c in range(4):
                pst = ps_setup.tile([128, 128], F32, tag="setup")
                nc.tensor.transpose(pst, ws_sb[:, r, c * 128 : (c + 1) * 128], ident)
                nc.scalar.copy(wsT_sb[:, c, r * 128 : (r + 1) * 128], pst)

        # ---- h_t^T: htT_sb[p, c, b] = h_t[b, c*128+p]
        htT_sb = singles.tile([128, 4, BL], F32)
        for c in range(4):
            pst = ps_setup.tile([128, BL], F32, tag="setup")
            nc.tensor.transpose(
                pst, ht_sb[:, c * 128 : (c + 1) * 128], ident[:BL, :BL]
            )
            nc.scalar.copy(htT_sb[:, c, :], pst)

        # ---- u[b] = W_score @ h_t[b] for all local batches ---------------
        # u_sb[p, hc, b] = sum_k W_score[hc*128+p, k] * h_t[b, k]
        u_sb = singles.tile([128, 4, BL], F32)
        for hc in range(4):
            psu = ps_setup.tile([128, BL], F32, tag="setup")
            for kc in range(4):
                nc.tensor.matmul(
                    psu,
                    lhsT=wsT_sb[:, kc, hc * 128 : (hc + 1) * 128],
                    rhs=htT_sb[:, kc, :],
                    start=(kc == 0),
                    stop=(kc == 3),
                )
            nc.scalar.copy(u_sb[:, hc, :], psu)

        # cast u to bf16 and stage u^T to DRAM for per-batch broadcast
        u_sb_bf = singles.tile([128, 4, BL], BF16)
        nc.vector.tensor_copy(out=u_sb_bf, in_=u_sb)
        for hc in range(4):
            uT_ap = bass.AP(
                tensor=uT_dram.tensor, offset=hc * 128, ap=[[1, 128], [H, BL]]
            )
            nc.sync.dma_start(out=uT_ap, in_=u_sb_bf[:, hc, :])

        # preT_sb[p, c, b]: transposed concat(context, h_t); ht half now
        preT_sb = singles.tile([128, 8, BL], F32)
        for c in range(4):
            nc.vector.tensor_copy(out=preT_sb[:, 4 + c, :], in_=htT_sb[:, c, :])

        # ---- main per-batch streaming loop ------------------------------
        NG = 4  # t-tile groups per batch; each group holds 4 t-tiles
        for b in range(BL):
            # u[b] (bf16) broadcast to all 128 partitions x 4 tile-slots
            u_bc = work.tile([128, 4, H], BF16, tag="u_bc")
            nc.gpsimd.dma_start(
                out=u_bc,
                in_=bass.AP(
                    tensor=uT_dram.tensor,
                    offset=b * H,
                    ap=[[0, 128], [0, 4], [1, H]],
                ),
            )

            S = small.tile([128, NT], F32, tag="S")
            hid_bfs = []
            for g in range(NG):
                hid4 = hid_pool.tile([128, 4, H], F32, tag="hid")
                nc.sync.dma_start(
                    out=hid4, in_=hs[b, g * 512 : (g + 1) * 512, :].rearrange(
                        "(n p) h -> p n h", p=128
                    )
                )
                hid4_bf = hid_bf_pool.tile([128, 4, H], BF16, tag="hidbf")
                nc.vector.tensor_copy(out=hid4_bf, in_=hid4)
                hid_bfs.append(hid4_bf)
                prod = work.tile([128, 4, H], BF16, tag="prod")
                nc.vector.tensor_mul(prod, hid4_bf, u_bc)
                dump = work.tile([128, H], BF16, tag="dump")
                for j in range(4):
                    # score column via ScalarE: S[:, 4g+j] = sum_h prod[:, j, :]
                    nc.scalar.activation(
                        dump,
                        prod[:, j, :],
                        mybir.ActivationFunctionType.Copy,
                        accum_out=S[:, 4 * g + j : 4 * g + j + 1],
                    )

            # softmax over all 2048 scores of this batch
            m_row = small.tile([128, 1], F32, tag="m_row")
            nc.vector.reduce_max(m_row, S, axis=mybir.AxisListType.X)
            M_all = small.tile([128, 1], F32, tag="M_all")
            nc.gpsimd.partition_all_reduce(
                M_all, m_row, channels=128, reduce_op=bass_isa.ReduceOp.max
            )
            nm = small.tile([128, 1], F32, tag="nm")
            nc.vector.tensor_scalar_mul(nm, M_all, -1.0)
            P = small.tile([128, NT], BF16, tag="P")
            l_row = small.tile([128, 1], F32, tag="l_row")
            nc.scalar.activation(
                P,
                S,
                mybir.ActivationFunctionType.Exp,
                bias=nm,
                scale=1.0,
                accum_out=l_row,
            )
            L_all = small.tile([128, 1], F32, tag="L_all")
            nc.gpsimd.partition_all_reduce(
                L_all, l_row, channels=128, reduce_op=bass_isa.ReduceOp.add
            )
            Linv = small.tile([128, 1], F32, tag="Linv")
            nc.vector.reciprocal(Linv, L_all)

            # context accumulation, already transposed:
            # psum_ctx[p, hc] = sum_t P[t] * hidden[t, hc*128+p]
            # (bf16 hidden chunk stationary, P column moving)
            psum_ctx = ps_ctx.tile([128, 4], F32, tag="ctx")
            for hc in range(4):
                for g in range(NG):
                    for j in range(4):
                        i = 4 * g + j
                        nc.tensor.matmul(
                            psum_ctx[:, hc : hc + 1],
                            lhsT=hid_bfs[g][:, j, hc * 128 : (hc + 1) * 128],
                            rhs=P[:, i : i + 1],
                            start=(i == 0),
                            stop=(i == NT - 1),
                        )
            # normalize by 1/L and write into preT layout
            nc.vector.tensor_scalar_mul(preT_sb[:, 0:4, b], psum_ctx, Linv)

        # ---- final: out = tanh(pre @ W_out) -----------------------------
        psum_out = ps_setup.tile([BL, UNITS], F32, tag="setup")
        for c in range(8):
            nc.tensor.matmul(
                psum_out,
                lhsT=preT_sb[:, c, :],
                rhs=wout_sb[:, c, :],
                start=(c == 0),
                stop=(c == 7),
            )
        y_sb = small.tile([BL, UNITS], F32, tag="y")
        nc.scalar.activation(y_sb, psum_out, mybir.ActivationFunctionType.Tanh)
        nc.sync.dma_start(out=out, in_=y_sb)


def build_nc():
    nc = bacc.Bacc(
        "TRN2",
        target_bir_lowering=False,
        debug=False,
        enable_asserts=False,
        num_devices=NCORES,
    )
    hs = nc.dram_tensor(
        "hidden_states", [BL, T, H], F32, kind="ExternalInput"
    ).ap()
    ws = nc.dram_tensor("W_score", [H, H], F32, kind="ExternalInput").ap()
    wo = nc.dram_tensor("W_out", [2 * H, UNITS], F32, kind="ExternalInput").ap()
    out = nc.dram_tensor("out", [BL, UNITS], F32, kind="ExternalOutput").ap()
    uT_dram = nc.dram_tensor("uT_scratch", [BL, H], BF16).ap()

    with tile.TileContext(nc) as tc:
        _kernel_body(tc, out, hs, ws, wo, uT_dram)
    nc.compile()
    return nc


_NC = None


def _get_nc():
    global _NC
    if _NC is None:
        _NC = build_nc()
    return _NC


def make_in_maps(hidden_states, W_score, W_out):
    hidden_states = np.ascontiguousarray(
        np.asarray(hidden_states, dtype=np.float32)
    )
    W_score = np.ascontiguousarray(np.asarray(W_score, dtype=np.float32))
    W_out = np.ascontiguousarray(np.asarray(W_out, dtype=np.float32))
    return [
        {
            "hidden_states": hidden_states[i * BL : (i + 1) * BL],
            "W_score": W_score,
            "W_out": W_out,
        }
        for i in range(NCORES)
    ]


def kernel(hidden_states, W_score, W_out):
    nc = _get_nc()
    in_maps = make_in_maps(hidden_states, W_score, W_out)
    res = run_bass_kernel_spmd(nc, in_maps, core_ids=list(range(NCORES)))
    return np.concatenate([res.results[i]["out"] for i in range(NCORES)], axis=0)


# revision 10
# speedup vs baseline: 2.0341x; 1.3527x over previous
"""Trainium2 Bass kernel for nn_Attention (pooling attention head).

Reference computation (per batch b):
    score[t]  = hidden[t,:] @ W_score @ hidden[-1,:]        # via u = W_score @ h_t
    attn      = softmax(score)
    context   = sum_t attn[t] * hidden[t,:]
    out       = tanh(concat(context, h_t) @ W_out)

Key optimization: the reference computes (hidden @ W_score) [B,T,H] first
(69 GFLOP); we reassociate to u = W_score @ h_t (34 MFLOP) and then
score = hidden @ u, so the kernel is a single memory-bound streaming pass
over hidden_states.

Sharding: data-parallel over batch, 8 batches per NeuronCore, no
collectives. Each core returns its [8, 128] slice of the output.
"""

import os

os.environ.setdefault("MYCRO_LOCAL_CACHE", "1")

from contextlib import ExitStack

import numpy as np

import concourse.bass as bass
import concourse.bass_isa as bass_isa
import concourse.tile as tile
from concourse import bacc, mybir
from concourse.bass_utils import run_bass_kernel_spmd
from concourse.masks import make_identity

B, T, H, UNITS = 64, 2048, 512, 128
NCORES = 8
BL = B // NCORES  # local batches per core
NT = T // 128  # 16 t-tiles per batch

F32 = mybir.dt.float32
F32R = mybir.dt.float32r
BF16 = mybir.dt.bfloat16

# bisection/debug knobs (comma-separated in KVAR):
#   nopar   — replace gpsimd.partition_all_reduce with a (numerically wrong) copy
#   ttr     — use fused tensor_tensor_reduce (FAILS on HW via axon path)
#   nobcast — replace the step-0 broadcast DMA with a (wrong) plain DMA
KVAR = set(os.environ.get("KVAR", "").split(",")) - {""}


def _kernel_body(tc: tile.TileContext, out, hs, ws, wo, uT_dram):
    nc = tc.nc
    with ExitStack() as ctx:
        singles = ctx.enter_context(tc.tile_pool(name="singles", bufs=1))
        hid_pool = ctx.enter_context(tc.tile_pool(name="hid", bufs=3))
        work = ctx.enter_context(tc.tile_pool(name="work", bufs=4))
        small = ctx.enter_context(tc.tile_pool(name="small", bufs=2))
        ps_setup = ctx.enter_context(tc.tile_pool(name="ps_setup", bufs=2, space="PSUM"))
        ps_ctx = ctx.enter_context(tc.tile_pool(name="ps_ctx", bufs=2, space="PSUM"))

        ident = singles.tile([128, 128], F32)
        make_identity(nc, ident)

        # ---- load weights / last-timestep rows --------------------------
        ws_sb = singles.tile([128, 4, H], F32)  # W_score rows r*128+p
        nc.sync.dma_start(out=ws_sb, in_=ws.rearrange("(r p) k -> p r k", p=128))
        wout_sb = singles.tile([128, 8, UNITS], F32)  # W_out rows c*128+p
        nc.sync.dma_start(out=wout_sb, in_=wo.rearrange("(c p) j -> p c j", p=128))
        ht_sb = singles.tile([BL, H], F32)  # h_t = hidden[:, -1, :]
        nc.sync.dma_start(out=ht_sb, in_=hs[:, T - 1, :])

        # ---- W_score^T (PE transposes): wsT_sb[p, kc, h] = W_score[h, kc*128+p]
        wsT_sb = singles.tile([128, 4, H], F32)
        for r in range(4):
            for c in range(4):
                pst = ps_setup.tile([128, 128], F32, tag="setup")
                nc.tensor.transpose(pst, ws_sb[:, r, c * 128 : (c + 1) * 128], ident)
                nc.scalar.copy(wsT_sb[:, c, r * 128 : (r + 1) * 128], pst)

        # ---- h_t^T: htT_sb[p, c, b] = h_t[b, c*128+p]
        htT_sb = singles.tile([128, 4, BL], F32)
        for c in range(4):
            pst = ps_setup.tile([128, BL], F32, tag="setup")
            nc.tensor.transpose(
                pst, ht_sb[:, c * 128 : (c + 1) * 128], ident[:BL, :BL]
            )
            nc.scalar.copy(htT_sb[:, c, :], pst)

        # ---- u[b] = W_score @ h_t[b] for all local batches ---------------
        # u_sb[p, hc, b] = sum_k W_score[hc*128+p, k] * h_t[b, k]
        u_sb = singles.tile([128, 4, BL], F32)
        for hc in range(4):
            psu = ps_setup.tile([128, BL], F32, tag="setup")
            for kc in range(4):
                nc.tensor.matmul(
                    psu,
                    lhsT=wsT_sb[:, kc, hc * 128 : (hc + 1) * 128],
                    rhs=htT_sb[:, kc, :],
                    start=(kc == 0),
                    stop=(kc == 3),
                )
            nc.scalar.copy(u_sb[:, hc, :], psu)

        # cast u to bf16 and stage u^T to DRAM for per-batch broadcast
        u_sb_bf = singles.tile([128, 4, BL], BF16)
        nc.vector.tensor_copy(out=u_sb_bf, in_=u_sb)
        for hc in range(4):
            uT_ap = bass.AP(
                tensor=uT_dram.tensor, offset=hc * 128, ap=[[1, 128], [H, BL]]
            )
            nc.sync.dma_start(out=uT_ap, in_=u_sb_bf[:, hc, :])

        # preT_sb[p, c, b]: transposed concat(context, h_t); ht half now
        preT_sb = singles.tile([128, 8, BL], F32)
        for c in range(4):
            nc.vector.tensor_copy(out=preT_sb[:, 4 + c, :], in_=htT_sb[:, c, :])

        # ---- main per-batch streaming loop ------------------------------
        # Layout: partition p holds t-rows p*16 .. p*16+15 (32KB contiguous
        # HBM reads per partition); column j of S/P maps to t = p*16 + j.
        # The softmax is order-agnostic and the PE contraction sums over all
        # (p, j), so the remapping is transparent.
        for b in range(BL):
            # u[b] (bf16) broadcast to all 128 partitions
            u_bc = work.tile([128, H], BF16, tag="u_bc")
            nc.gpsimd.dma_start(
                out=u_bc,
                in_=bass.AP(
                    tensor=uT_dram.tensor,
                    offset=b * H,
                    ap=[[0, 128], [1, H]],
                ),
            )

            # whole-batch load with inline fp32->bf16 cast (SWDGE)
            hid_bf = hid_pool.tile([128, NT, H], BF16, tag="hid")
            nc.gpsimd.dma_start(
                out=hid_bf,
                in_=hs[b].rearrange("(p n) h -> p n h", p=128),
            )

            S = small.tile([128, NT], F32, tag="S")
            dump = work.tile([128, H], BF16, tag="dump")
            for j in range(NT):
                prod = work.tile([128, H], BF16, tag="prod")
                nc.vector.tensor_mul(prod, hid_bf[:, j, :], u_bc)
                # score column via ScalarE: S[:, j] = sum_h prod
                nc.scalar.activation(
                    dump,
                    prod,
                    mybir.ActivationFunctionType.Copy,
                    accum_out=S[:, j : j + 1],
                )

            # softmax over all 2048 scores of this batch
            m_row = small.tile([128, 1], F32, tag="m_row")
            nc.vector.reduce_max(m_row, S, axis=mybir.AxisListType.X)
            M_all = small.tile([128, 1], F32, tag="M_all")
            nc.gpsimd.partition_all_reduce(
                M_all, m_row, channels=128, reduce_op=bass_isa.ReduceOp.max
            )
            nm = small.tile([128, 1], F32, tag="nm")
            nc.vector.tensor_scalar_mul(nm, M_all, -1.0)
            P = small.tile([128, NT], BF16, tag="P")
            l_row = small.tile([128, 1], F32, tag="l_row")
            nc.scalar.activation(
                P,
                S,
                mybir.ActivationFunctionType.Exp,
                bias=nm,
                scale=1.0,
                accum_out=l_row,
            )
            L_all = small.tile([128, 1], F32, tag="L_all")
            nc.gpsimd.partition_all_reduce(
                L_all, l_row, channels=128, reduce_op=bass_isa.ReduceOp.add
            )
            Linv = small.tile([128, 1], F32, tag="Linv")
            nc.vector.reciprocal(Linv, L_all)

            # context accumulation, already transposed:
            # psum_ctx[p_h, hc] = sum_t P[t] * hidden[t, hc*128+p_h]
            psum_ctx = ps_ctx.tile([128, 4], F32, tag="ctx")
            for hc in range(4):
                for j in range(NT):
                    nc.tensor.matmul(
                        psum_ctx[:, hc : hc + 1],
                        lhsT=hid_bf[:, j, hc * 128 : (hc + 1) * 128],
                        rhs=P[:, j : j + 1],
                        start=(j == 0),
                        stop=(j == NT - 1),
                    )
            # normalize by 1/L and write into preT layout
            nc.vector.tensor_scalar_mul(preT_sb[:, 0:4, b], psum_ctx, Linv)

        # ---- final: out = tanh(pre @ W_out) -----------------------------
        psum_out = ps_setup.tile([BL, UNITS], F32, tag="setup")
        for c in range(8):
            nc.tensor.matmul(
                psum_out,
                lhsT=preT_sb[:, c, :],
                rhs=wout_sb[:, c, :],
                start=(c == 0),
                stop=(c == 7),
            )
        y_sb = small.tile([BL, UNITS], F32, tag="y")
        nc.scalar.activation(y_sb, psum_out, mybir.ActivationFunctionType.Tanh)
        nc.sync.dma_start(out=out, in_=y_sb)


def build_nc():
    nc = bacc.Bacc(
        "TRN2",
        target_bir_lowering=False,
        debug=False,
        enable_asserts=False,
        num_devices=NCORES,
    )
    hs = nc.dram_tensor(
        "hidden_states", [BL, T, H], F32, kind="ExternalInput"
    ).ap()
    ws = nc.dram_tensor("W_score", [H, H], F32, kind="ExternalInput").ap()
    wo = nc.dram_tensor("W_out", [2 * H, UNITS], F32, kind="ExternalInput").ap()
    out = nc.dram_tensor("out", [BL, UNITS], F32, kind="ExternalOutput").ap()
    uT_dram = nc.dram_tensor("uT_scratch", [BL, H], BF16).ap()

    with tile.TileContext(nc) as tc:
        _kernel_body(tc, out, hs, ws, wo, uT_dram)
    nc.compile()
    return nc


_NC = None


def _get_nc():
    global _NC
    if _NC is None:
        _NC = build_nc()
    return _NC


def make_in_maps(hidden_states, W_score, W_out):
    hidden_states = np.ascontiguousarray(
        np.asarray(hidden_states, dtype=np.float32)
    )
    W_score = np.ascontiguousarray(np.asarray(W_score, dtype=np.float32))
    W_out = np.ascontiguousarray(np.asarray(W_out, dtype=np.float32))
    return [
        {
            "hidden_states": hidden_states[i * BL : (i + 1) * BL],
            "W_score": W_score,
            "W_out": W_out,
        }
        for i in range(NCORES)
    ]


def kernel(hidden_states, W_score, W_out):
    nc = _get_nc()
    in_maps = make_in_maps(hidden_states, W_score, W_out)
    res = run_bass_kernel_spmd(nc, in_maps, core_ids=list(range(NCORES)))
    return np.concatenate([res.results[i]["out"] for i in range(NCORES)], axis=0)


# revision 19
# speedup vs baseline: 2.2339x; 1.0982x over previous
"""Trainium2 Bass kernel for nn_Attention (pooling attention head).

Reference computation (per batch b):
    score[t]  = hidden[t,:] @ W_score @ hidden[-1,:]        # via u = W_score @ h_t
    attn      = softmax(score)
    context   = sum_t attn[t] * hidden[t,:]
    out       = tanh(concat(context, h_t) @ W_out)

Key optimization: the reference computes (hidden @ W_score) [B,T,H] first
(69 GFLOP); we reassociate to u = W_score @ h_t (34 MFLOP) and then
score = hidden @ u, so the kernel is a single memory-bound streaming pass
over hidden_states.

Sharding: data-parallel over batch, 8 batches per NeuronCore, no
collectives. Each core returns its [8, 128] slice of the output.

Layout: partition p holds t-rows p*16 .. p*16+15 (16-32KB contiguous HBM
reads per partition); column j of S/P maps to t = p*16 + j. The softmax
is order-agnostic and the PE contraction sums over all (p, j), so the
remapping is transparent.

Engine budget per batch (~11us each, "ridge"):
  SWDGE ring: 2x 2MB cast-DMA (fp32->bf16 inline) + u broadcast
  DVE:        16x bf16 mul (2x mode) + 3 reductions + small softmax ops
  ACT:        13x copy-accum score reductions + exp
  PE:         64x (ldweights + N=1 matmul) context accumulation + stats
"""

import os

os.environ.setdefault("MYCRO_LOCAL_CACHE", "1")

from contextlib import ExitStack

import numpy as np

import concourse.bass as bass
import concourse.tile as tile
from concourse import bacc, mybir
from concourse.bass_utils import run_bass_kernel_spmd
from concourse.masks import make_identity

B, T, H, UNITS = 64, 2048, 512, 128
NCORES = 8
BL = B // NCORES  # local batches per core
NT = T // 128  # 16 t-tiles per batch

F32 = mybir.dt.float32
BF16 = mybir.dt.bfloat16

# which score reductions run on DVE instead of ACT (load balancing)
DVE_REDUCE_COLS = (5, 10, 15)


def _kernel_body(tc: tile.TileContext, out, hs, ws, wo):
    nc = tc.nc
    with ExitStack() as ctx:
        singles = ctx.enter_context(tc.tile_pool(name="singles", bufs=1))
        hid_pool = ctx.enter_context(tc.tile_pool(name="hid", bufs=8))
        work = ctx.enter_context(tc.tile_pool(name="work", bufs=4))
        small = ctx.enter_context(tc.tile_pool(name="small", bufs=2))
        ps_setup = ctx.enter_context(
            tc.tile_pool(name="ps_setup", bufs=2, space="PSUM")
        )
        ps_ctx = ctx.enter_context(tc.tile_pool(name="ps_ctx", bufs=2, space="PSUM"))
        ps_stat = ctx.enter_context(tc.tile_pool(name="ps_stat", bufs=2, space="PSUM"))
        dram = ctx.enter_context(tc.tile_pool(name="dram", bufs=1, space="DRAM"))

        uT_dram = dram.tile([BL, H], BF16)
        ident = singles.tile([128, 128], F32)
        make_identity(nc, ident)

        # ---- load weights / last-timestep rows --------------------------
        ws_sb = singles.tile([128, 4, H], F32)  # W_score rows r*128+p
        nc.sync.dma_start(out=ws_sb, in_=ws.rearrange("(r p) k -> p r k", p=128))
        wout_sb = singles.tile([128, 8, UNITS], F32)  # W_out rows c*128+p
        nc.sync.dma_start(out=wout_sb, in_=wo.rearrange("(c p) j -> p c j", p=128))
        ht_sb = singles.tile([BL, H], F32)  # h_t = hidden[:, -1, :]
        nc.sync.dma_start(out=ht_sb, in_=hs[:, T - 1, :])

        # ---- W_score^T (PE transposes): wsT_sb[p, kc, h] = W_score[h, kc*128+p]
        wsT_sb = singles.tile([128, 4, H], F32)
        for r in range(4):
            for c in range(4):
                pst = ps_setup.tile([128, 128], F32, tag="setup")
                nc.tensor.transpose(pst, ws_sb[:, r, c * 128 : (c + 1) * 128], ident)
                nc.scalar.copy(wsT_sb[:, c, r * 128 : (r + 1) * 128], pst)

        # ---- h_t^T: htT_sb[p, c, b] = h_t[b, c*128+p]
        htT_sb = singles.tile([128, 4, BL], F32)
        for c in range(4):
            pst = ps_setup.tile([128, BL], F32, tag="setup")
            nc.tensor.transpose(
                pst, ht_sb[:, c * 128 : (c + 1) * 128], ident[:BL, :BL]
            )
            nc.scalar.copy(htT_sb[:, c, :], pst)

        # ---- u[b] = W_score @ h_t[b] for all local batches ---------------
        # u_sb[p, hc, b] = sum_k W_score[hc*128+p, k] * h_t[b, k]
        u_sb = singles.tile([128, 4, BL], F32)
        for hc in range(4):
            psu = ps_setup.tile([128, BL], F32, tag="setup")
            for kc in range(4):
                nc.tensor.matmul(
                    psu,
                    lhsT=wsT_sb[:, kc, hc * 128 : (hc + 1) * 128],
                    rhs=htT_sb[:, kc, :],
                    start=(kc == 0),
                    stop=(kc == 3),
                )
            nc.scalar.copy(u_sb[:, hc, :], psu)

        # cast u to bf16 and stage u^T to DRAM for per-batch broadcast
        u_sb_bf = singles.tile([128, 4, BL], BF16)
        nc.vector.tensor_copy(out=u_sb_bf, in_=u_sb)
        uT_view = uT_dram.rearrange("b (c p) -> p c b", p=128)
        for hc in range(4):
            nc.sync.dma_start(out=uT_view[:, hc, :], in_=u_sb_bf[:, hc, :])

        # preT_sb[p, c, b]: transposed concat(context, h_t); ht half now
        preT_sb = singles.tile([128, 8, BL], F32)
        for c in range(4):
            nc.vector.tensor_copy(out=preT_sb[:, 4 + c, :], in_=htT_sb[:, c, :])

        # ones row for PE-based partition broadcasts
        ones_sb = singles.tile([1, 128], F32)
        nc.vector.memset(ones_sb, 1.0)

        # ---- main per-batch streaming loop ------------------------------
        for b in range(BL):
            # u[b] broadcast to all partitions (whole-tile write: tracked)
            u_bc = work.tile([128, H], BF16, tag="u_bc")
            nc.gpsimd.dma_start(
                out=u_bc,
                in_=uT_dram[b : b + 1, :].to_broadcast([128, H]),
            )

            # whole-batch load with inline fp32->bf16 cast (SWDGE),
            # split in two halves for finer pipelining
            hs_v = hs[b].rearrange("(p n) h -> p n h", p=128)
            hid_halves = []
            for half in range(2):
                hid_bf = hid_pool.tile([128, NT // 2, H], BF16, tag="hid")
                nc.gpsimd.dma_start(
                    out=hid_bf, in_=hs_v[:, half * 8 : half * 8 + 8, :]
                )
                hid_halves.append(hid_bf)

            S = small.tile([128, NT], F32, tag="S")
            dump = work.tile([128, H], BF16, tag="dump")
            for j in range(NT):
                src = hid_halves[j // 8][:, j % 8, :]
                prod = work.tile([128, H], BF16, tag="prod")
                nc.vector.tensor_mul(prod, src, u_bc)
                if j in DVE_REDUCE_COLS:
                    nc.vector.reduce_sum(
                        S[:, j : j + 1], prod, axis=mybir.AxisListType.X
                    )
                else:
                    # score column via ScalarE: S[:, j] = sum_h prod
                    nc.scalar.activation(
                        dump,
                        prod,
                        mybir.ActivationFunctionType.Copy,
                        accum_out=S[:, j : j + 1],
                    )

            # softmax over all 2048 scores; cross-partition stats via PE
            # (transpose + ones-matmul broadcast) to keep gpsimd DMA-only
            m_row = small.tile([128, 1], F32, tag="m_row")
            nc.vector.reduce_max(m_row, S, axis=mybir.AxisListType.X)
            mT_ps = ps_stat.tile([1, 128], F32, tag="stat")
            nc.tensor.transpose(mT_ps, m_row, ident)
            M_sb = small.tile([1, 1], F32, tag="M_sb")
            nc.vector.reduce_max(M_sb, mT_ps[0:1, :], axis=mybir.AxisListType.X)
            Mb_ps = ps_stat.tile([128, 1], F32, tag="stat")
            nc.tensor.matmul(Mb_ps, lhsT=ones_sb, rhs=M_sb, start=True, stop=True)
            nm = small.tile([128, 1], F32, tag="nm")
            nc.vector.tensor_scalar_mul(nm, Mb_ps, -1.0)

            P = small.tile([128, NT], BF16, tag="P")
            l_row = small.tile([128, 1], F32, tag="l_row")
            nc.scalar.activation(
                P,
                S,
                mybir.ActivationFunctionType.Exp,
                bias=nm,
                scale=1.0,
                accum_out=l_row,
            )
            lT_ps = ps_stat.tile([1, 128], F32, tag="stat")
            nc.tensor.transpose(lT_ps, l_row, ident)
            L_sb = small.tile([1, 1], F32, tag="L_sb")
            nc.vector.reduce_sum(L_sb, lT_ps[0:1, :], axis=mybir.AxisListType.X)
            Linv_sb = small.tile([1, 1], F32, tag="Linv_sb")
            nc.vector.reciprocal(Linv_sb, L_sb)
            Lb_ps = ps_stat.tile([128, 1], F32, tag="stat")
            nc.tensor.matmul(Lb_ps, lhsT=ones_sb, rhs=Linv_sb, start=True, stop=True)
            Linv = small.tile([128, 1], F32, tag="Linv")
            nc.vector.tensor_copy(out=Linv, in_=Lb_ps)

            # context accumulation, already transposed:
            # psum_ctx[p_h, hc] = sum_t P[t] * hidden[t, hc*128+p_h]
            # (bf16 hidden chunk stationary, P column moving)
            psum_ctx = ps_ctx.tile([128, 4], F32, tag="ctx")
            for hc in range(4):
                for j in range(NT):
                    nc.tensor.matmul(
                        psum_ctx[:, hc : hc + 1],
                        lhsT=hid_halves[j // 8][
                            :, j % 8, hc * 128 : (hc + 1) * 128
                        ],
                        rhs=P[:, j : j + 1],
                        start=(j == 0),
                        stop=(j == NT - 1),
                    )
            # normalize by 1/L and write into preT layout
            nc.vector.tensor_scalar_mul(preT_sb[:, 0:4, b], psum_ctx, Linv)

        # ---- final: out = tanh(pre @ W_out) -----------------------------
        psum_out = ps_setup.tile([BL, UNITS], F32, tag="setup")
        for c in range(8):
            nc.tensor.matmul(
                psum_out,
                lhsT=preT_sb[:, c, :],
                rhs=wout_sb[:, c, :],
                start=(c == 0),
                stop=(c == 7),
            )
        y_sb = small.tile([BL, UNITS], F32, tag="y")
        nc.scalar.activation(y_sb, psum_out, mybir.ActivationFunctionType.Tanh)
        nc.sync.dma_start(out=out, in_=y_sb)


def build_nc():
    nc = bacc.Bacc(
        "TRN2",
        target_bir_lowering=False,
        debug=False,
        enable_asserts=False,
        num_devices=NCORES,
    )
    hs = nc.dram_tensor(
        "hidden_states", [BL, T, H], F32, kind="ExternalInput"
    ).ap()
    ws = nc.dram_tensor("W_score", [H, H], F32, kind="ExternalInput").ap()
    wo = nc.dram_tensor("W_out", [2 * H, UNITS], F32, kind="ExternalInput").ap()
    out = nc.dram_tensor("out", [BL, UNITS], F32, kind="ExternalOutput").ap()

    with tile.TileContext(nc) as tc:
        _kernel_body(tc, out, hs, ws, wo)
    nc.compile()
    return nc


_NC = None


def _get_nc():
    global _NC
    if _NC is None:
        _NC = build_nc()
    return _NC


def make_in_maps(hidden_states, W_score, W_out):
    hidden_states = np.ascontiguousarray(
        np.asarray(hidden_states, dtype=np.float32)
    )
    W_score = np.ascontiguousarray(np.asarray(W_score, dtype=np.float32))
    W_out = np.ascontiguousarray(np.asarray(W_out, dtype=np.float32))
    return [
        {
            "hidden_states": hidden_states[i * BL : (i + 1) * BL],
            "W_score": W_score,
            "W_out": W_out,
        }
        for i in range(NCORES)
    ]


def kernel(hidden_states, W_score, W_out):
    nc = _get_nc()
    in_maps = make_in_maps(hidden_states, W_score, W_out)
    res = run_bass_kernel_spmd(nc, in_maps, core_ids=list(range(NCORES)))
    return np.concatenate([res.results[i]["out"] for i in range(NCORES)], axis=0)
